# revision 3
# baseline (speedup 1.0000x reference)
"""Two-layer DGL-style GraphConv (norm='both') + PReLU on 8 TRN2 NeuronCores.

Strategy (dst-sharded graph parallel, per the sharding hint):
  - nodes are split into 8 contiguous ranges of 12500; core k owns range k
    (its segment_sum destination rows and its output rows).
  - edges are routed to the core owning their dst, grouped into windows of
    WIN=128 destination rows, and within a window grouped by src chunk of
    32768 rows (dma_gather's int16 index limit).
  - per core the full (replicated) feature table lives in HBM; message rows
    h[src] are fetched with gpsimd.dma_gather (128 rows per tile column).
  - aggregation is a one-hot matmul: S[e, d] = (iota[d]==dst_local[e])*coef[e]
    built on-chip with one fused tensor_scalar; psum[f, d] += H[e, f].T @ S.
    coef[e] = dout_is[src]*din_is[dst] folds both degree normalizations in
    (degrees are a host-side byproduct of edge partitioning).
  - window epilogue: m.T -> SBUF, out[d, j] = (m @ W) + b (bias as a K=1
    matmul), PReLU = relu(z) - a ⊙ relu(-z), DMA rows to the layer table.
  - AllGather shares layer-1 shards to every core for the second layer.
"""
import sys

import numpy as np

sys.path.insert(0, '/opt/trn_rl_repo')
import concourse.bacc as bacc
import concourse.mybir as mybir
from concourse import tile
from concourse.bass_utils import run_bass_kernel_spmd

F32 = mybir.dt.float32
I16 = mybir.dt.int16
AF = mybir.ActivationFunctionType
AL = mybir.AluOpType

P = 128
CHUNK = 32768

N_NODES = 100000
N_EDGES = 3200000
N_CORES = 8
WIN = 128
GROUP = 2

_waitfix_ctr = [0]


def split_multi_waits(nc):
    """This walrus accepts only ONE sync-wait command on several ISA structs
    (Drain, extended DMA gather, ...). Hoist extras onto InstEventSemaphore
    carriers placed just before the instruction. Run after nc.finalize()."""
    n_fixed = 0
    for fn in nc.m.functions:
        for bb in fn.blocks:
            insts = list(bb.instructions)
            out = []
            changed = False
            for inst in insts:
                si = inst.sync_info
                if si is not None and si.on_wait is not None and len(si.on_wait) > 1:
                    waits = list(si.on_wait)
                    for w in waits[:-1]:
                        _waitfix_ctr[0] += 1
                        ev = mybir.InstEventSemaphore(
                            name=f"I-waitfix-{_waitfix_ctr[0]}", ins=[], outs=[])
                        ev.engine = inst.engine
                        ev.sync_info = mybir.SyncInfo(on_wait=[w], on_update=[])
                        nc.register_instruction(ev)
                        out.append(ev)
                    si.on_wait = [waits[-1]]
                    n_fixed += 1
                    changed = True
                out.append(inst)
            if changed:
                bb.instructions[:] = out
    return n_fixed


def preprocess(edge_index, n_nodes, n_cores, win, group_sz, chunk=CHUNK):
    src = np.asarray(edge_index[0]).astype(np.int64)
    dst = np.asarray(edge_index[1]).astype(np.int64)
    deg_out = np.bincount(src, minlength=n_nodes).astype(np.float32)
    deg_in = np.bincount(dst, minlength=n_nodes).astype(np.float32)
    dout_is = 1.0 / np.sqrt(np.maximum(deg_out, 1.0))
    din_is = 1.0 / np.sqrt(np.maximum(deg_in, 1.0))
    coef = (dout_is[src] * din_is[dst]).astype(np.float32)

    npc = n_nodes // n_cores
    nwin = (npc + win - 1) // win
    nch = (n_nodes + chunk - 1) // chunk
    core = dst // npc
    dl = dst % npc
    w = dl // win
    dlw = (dl % win).astype(np.float32)
    ch = src // chunk

    key = (core * nwin + w) * nch + ch
    order = np.argsort(key, kind='stable')
    s_src = src[order]
    s_coef = coef[order]
    s_dlw = dlw[order]
    cnt = np.bincount(key[order], minlength=n_cores * nwin * nch)
    cnt = cnt.reshape(n_cores, nwin, nch)
    off = np.zeros_like(cnt)
    off.flat[1:] = np.cumsum(cnt.flat)[:-1]

    T = np.ceil(cnt.max(axis=0) / P).astype(np.int64)
    T = np.maximum(T, 1)

    ngrp = (nwin + group_sz - 1) // group_sz
    groups = [list(range(g * group_sz, min((g + 1) * group_sz, nwin)))
              for g in range(ngrp)]

    col0 = {}
    idx_off = {}
    num_idxs = {}
    grp_col0 = []
    grp_cols = []
    tot_cols = 0
    tot_idx = 0
    for g, ws in enumerate(groups):
        grp_col0.append(tot_cols)
        for c in range(nch):
            ni = int(sum(T[w_, c] for w_ in ws)) * P
            idx_off[(g, c)] = tot_idx
            num_idxs[(g, c)] = ni
            tot_idx += (ni // 16) * P
            for w_ in ws:
                col0[(g, c, w_)] = tot_cols
                tot_cols += int(T[w_, c])
        grp_cols.append(tot_cols - grp_col0[g])

    plan = dict(chunk=chunk, n_nodes=n_nodes, n_cores=n_cores, npc=npc,
                win=win, nwin=nwin, nch=nch, groups=groups, T=T, col0=col0,
                idx_off=idx_off, num_idxs=num_idxs, grp_col0=grp_col0,
                grp_cols=grp_cols, tot_cols=tot_cols, tot_idx=tot_idx)

    per_core = []
    for k in range(n_cores):
        dst2d = np.zeros((P, tot_cols), dtype=np.float32)
        coef2d = np.zeros((P, tot_cols), dtype=np.float32)
        idxflat = np.zeros(tot_idx, dtype=np.int16)
        for g, ws in enumerate(groups):
            for c in range(nch):
                ni = num_idxs[(g, c)]
                call_idx = np.zeros(ni, dtype=np.int64)
                qbase = 0
                for w_ in ws:
                    n = int(cnt[k, w_, c])
                    o = int(off[k, w_, c])
                    cb = col0[(g, c, w_)]
                    tcols = int(T[w_, c])
                    j = np.arange(n)
                    call_idx[qbase + j] = s_src[o:o + n] - c * chunk
                    dst2d[j % P, cb + j // P] = s_dlw[o:o + n]
                    coef2d[j % P, cb + j // P] = s_coef[o:o + n]
                    qbase += tcols * P
                a = call_idx.astype(np.int16).reshape(ni // 16, 16).T
                a = np.tile(a, (8, 1))
                io = idx_off[(g, c)]
                idxflat[io:io + a.size] = a.reshape(-1)
        per_core.append(dict(dst2d=dst2d, coef2d=coef2d, idxflat=idxflat))

    return plan, per_core


def build_nc(plan, dtype=F32):
    n_nodes = plan['n_nodes']
    n_cores = plan['n_cores']
    npc = plan['npc']
    win = plan['win']
    groups = plan['groups']
    T = plan['T']
    col0 = plan['col0']
    idx_off = plan['idx_off']
    num_idxs = plan['num_idxs']
    grp_col0 = plan['grp_col0']
    grp_cols = plan['grp_cols']
    D = 128

    nc = bacc.Bacc("TRN2", num_swdge_queues=4)
    feat = nc.declare_dram_parameter("features", [n_nodes, D], F32, isOutput=False)
    gidx = nc.declare_dram_parameter("gidx", [plan['tot_idx']], I16, isOutput=False)
    gdst = nc.declare_dram_parameter("gdst", [P, plan['tot_cols']], F32, isOutput=False)
    gcoef = nc.declare_dram_parameter("gcoef", [P, plan['tot_cols']], F32, isOutput=False)
    iota_in = nc.declare_dram_parameter("iota", [P, win], F32, isOutput=False)
    abc_in = nc.declare_dram_parameter("abc", [P, D], F32, isOutput=False)
    w1_in = nc.declare_dram_parameter("W1", [D, D], F32, isOutput=False)
    w2_in = nc.declare_dram_parameter("W2", [D, D], F32, isOutput=False)
    b1_in = nc.declare_dram_parameter("b1r", [1, D], F32, isOutput=False)
    b2_in = nc.declare_dram_parameter("b2r", [1, D], F32, isOutput=False)
    ones_in = nc.declare_dram_parameter("ones1", [1, D], F32, isOutput=False)
    out = nc.declare_dram_parameter("out", [npc, D], F32, isOutput=True)

    h1_shard = nc.dram_tensor("h1_shard", [npc, D], F32)
    h1_full = nc.dram_tensor("h1_full", [n_cores * npc, D], F32, addr_space="Shared")

    with tile.TileContext(nc) as tc:
        with (
            tc.tile_pool(name="const", bufs=1) as cpool,
            tc.tile_pool(name="meta", bufs=2) as mpool,
            tc.tile_pool(name="hbuf", bufs=2) as hpool,
            tc.tile_pool(name="sbuf", bufs=6) as spool,
            tc.tile_pool(name="epil", bufs=3) as epool,
            tc.tile_pool(name="pm", bufs=2, space="PSUM") as pmpool,
            tc.tile_pool(name="po", bufs=2, space="PSUM") as popool,
        ):
            iota_t = cpool.tile([P, win], F32)
            nc.sync.dma_start(out=iota_t[:], in_=iota_in[:])
            abc_t = cpool.tile([P, D], F32)
            nc.sync.dma_start(out=abc_t[:], in_=abc_in[:])
            w1_t = cpool.tile([D, D], F32)
            nc.sync.dma_start(out=w1_t[:], in_=w1_in[:])
            w2_t = cpool.tile([D, D], F32)
            nc.sync.dma_start(out=w2_t[:], in_=w2_in[:])
            b1_t = cpool.tile([1, D], F32)
            nc.sync.dma_start(out=b1_t[:], in_=b1_in[:])
            b2_t = cpool.tile([1, D], F32)
            nc.sync.dma_start(out=b2_t[:], in_=b2_in[:])
            ones_t = cpool.tile([1, D], F32)
            nc.sync.dma_start(out=ones_t[:], in_=ones_in[:])

            def layer(table_h, w_t, b_t, out_dram, out_rows_full):
                for g, ws in enumerate(groups):
                    gc0, gcc = grp_col0[g], grp_cols[g]
                    dst_t = mpool.tile([P, gcc], F32, tag="dstm")
                    nc.sync.dma_start(out=dst_t[:], in_=gdst[:, gc0:gc0 + gcc])
                    coef_t = mpool.tile([P, gcc], F32, tag="coefm")
                    nc.sync.dma_start(out=coef_t[:], in_=gcoef[:, gc0:gc0 + gcc])
                    hts = {}
                    for c in range(plan['nch']):
                        ni = num_idxs[(g, c)]
                        if ni == 0:
                            continue
                        io = idx_off[(g, c)]
                        it = mpool.tile([P, ni // 16], I16, tag=f"idxm{c}")
                        nc.sync.dma_start(
                            out=it[:],
                            in_=gidx[io:io + (ni // 16) * P].rearrange(
                                "(p c) -> p c", p=P))
                        ht = hpool.tile([P, (ni // P) * D], dtype, tag=f"hbuf{c}")
                        r0c = c * plan['chunk']
                        r1c = min((c + 1) * plan['chunk'], plan['n_nodes'])
                        nc.gpsimd.dma_gather(
                            ht[:].rearrange("p (t e) -> p t e", e=D),
                            table_h[r0c:r1c, :], it[:], ni, ni, D,
                            single_packet=False, queue_num=c % 4)
                        hts[c] = ht
                    for w_ in ws:
                        pm = pmpool.tile([P, win], F32, tag="pm")
                        first = True
                        for c in range(plan['nch']):
                            if num_idxs[(g, c)] == 0:
                                continue
                            tw = int(T[w_, c])
                            ht = hts[c]
                            lt0 = int(sum(T[w2_, c] for w2_ in ws if w2_ < w_))
                            cb = col0[(g, c, w_)]
                            for t in range(tw):
                                colg = cb + t
                                s_t = spool.tile([P, win], dtype, tag="sm")
                                nc.any.tensor_scalar(
                                    out=s_t[:], in0=iota_t[:],
                                    scalar1=dst_t[:, colg - gc0:colg - gc0 + 1],
                                    scalar2=coef_t[:, colg - gc0:colg - gc0 + 1],
                                    op0=AL.is_equal, op1=AL.mult)
                                lt = lt0 + t
                                nc.tensor.matmul(
                                    out=pm[:],
                                    lhsT=ht[:, (lt * D):(lt + 1) * D],
                                    rhs=s_t[:],
                                    start=first,
                                    stop=(c == plan['nch'] - 1 and t == tw - 1))
                                first = False
                        mt_sb = epool.tile([P, win], F32, tag="mts")
                        nc.scalar.copy(out=mt_sb[:], in_=pm[:])
                        po = popool.tile([win, D], F32, tag="po")
                        nc.tensor.matmul(out=po[:], lhsT=mt_sb[:], rhs=w_t[:],
                                         start=True, stop=False)
                        nc.tensor.matmul(out=po[:], lhsT=ones_t[:1, :win],
                                         rhs=b_t[:1, :], start=False, stop=True)
                        tpos = epool.tile([win, D], F32, tag="tpos")
                        nc.scalar.activation(tpos[:], po[:], AF.Relu)
                        tneg = epool.tile([win, D], F32, tag="tneg")
                        nc.scalar.activation(tneg[:], po[:], AF.Relu, scale=-1.0)
                        nc.vector.tensor_tensor(out=tneg[:], in0=tneg[:],
                                                in1=abc_t[:win, :], op=AL.mult)
                        ot = epool.tile([win, D], F32, tag="ot")
                        nc.vector.tensor_tensor(out=ot[:], in0=tpos[:],
                                                in1=tneg[:], op=AL.subtract)
                        r0 = w_ * win
                        rows = min(win, out_rows_full - r0)
                        nc.sync.dma_start(out=out_dram[r0:r0 + rows, :],
                                          in_=ot[:rows, :])

            layer(feat, w1_t[:], b1_t[:], h1_shard, npc)
            nc.gpsimd.collective_compute(
                "AllGather", AL.bypass,
                replica_groups=[list(range(n_cores))],
                ins=[h1_shard[:]], outs=[h1_full[:]])
            layer(h1_full, w2_t[:], b2_t[:], out, npc)

    nc.finalize()
    split_multi_waits(nc)
    return nc


def make_inputs(plan, per_core, features, W1, b1, W2, b2, prelu_a):
    win = plan['win']
    iota = np.tile(np.arange(win, dtype=np.float32), (P, 1))
    abc = np.tile(np.asarray(prelu_a, np.float32), (P, 1))
    ones1 = np.ones((1, 128), np.float32)
    feats = np.ascontiguousarray(np.asarray(features, np.float32))
    in_maps = []
    for k in range(plan['n_cores']):
        in_maps.append({
            "features": feats,
            "gidx": per_core[k]['idxflat'],
            "gdst": per_core[k]['dst2d'],
            "gcoef": per_core[k]['coef2d'],
            "iota": iota,
            "abc": abc,
            "W1": np.asarray(W1, np.float32),
            "W2": np.asarray(W2, np.float32),
            "b1r": np.asarray(b1, np.float32).reshape(1, -1),
            "b2r": np.asarray(b2, np.float32).reshape(1, -1),
            "ones1": ones1,
        })
    return in_maps


def _run(inputs, trace=False):
    import time as _time
    features = inputs["features"]
    edge_index = inputs["edge_index"]
    plan, per_core = preprocess(edge_index, N_NODES, N_CORES, WIN, GROUP)
    nc = build_nc(plan)
    in_maps = make_inputs(plan, per_core, features,
                          inputs["W1"], inputs["b1"], inputs["W2"],
                          inputs["b2"], inputs["prelu_a"])
    t0 = _time.perf_counter()
    res = run_bass_kernel_spmd(nc, in_maps, list(range(N_CORES)), trace=trace)
    t1 = _time.perf_counter()
    out = np.concatenate([res.results[k]["out"] for k in range(N_CORES)], axis=0)
    return out, res, t1 - t0


def kernel(**inputs) -> np.ndarray:
    out, _, _ = _run(inputs, trace=False)
    return out

